# revision 20
# baseline (speedup 1.0000x reference)
"""APQB attention kernel for 8 Trainium2 NeuronCores.

Sharding: core = 2*b + t_half (data parallel over batch and query rows).
Each core computes y[b, t0:t0+512, :] with zero cross-core communication.

Per-core device pipeline (all layouts chosen so no on-device transposes
are needed; host pre-transposes x and the weight matrices):
  qT = WqT.T @ xT[:, :512] + bq      [d=1024, t=512]   (fp32r matmuls)
  kT = WkT.T @ xT + bk               [d=1024, s=1024]
  v  = xT.T @ WvT + bv               [s=1024, d=1024]  (bf16 out)
  per head h:
    S_T  = kT_h.T @ qT_h             [s=1024, t=512]   (K=64 contraction)
    P    = exp(S_T*scale + bias_h)   (ScalarE, bf16 out)
    den  = ones.T @ P                (PE ones-matmul -> softmax denominator)
    Pm   = P * (noise_T > Tm)        (DVE mask gen + mask mul, bf16)
    OT_h = v_h.T @ Pm                [d=64, t=512] accumulated in PSUM
    OT_h *= 1/(den*(1-Tm+1e-8))     (per-pair DVE scale on PSUM->SBUF copy)
  y = [OT; ones].T @ [WoT; bo]       [t=512, f=1024]
"""

import numpy as np

try:
    import concourse.bass as bass
except ImportError:
    import sys
    sys.path.insert(0, "/opt/trn_rl_repo")
    import concourse.bass as bass

import concourse.tile as tile
from concourse import bacc, mybir
from concourse.bass_utils import run_bass_kernel_spmd
from concourse.tile_rust import add_dep_helper

F32 = mybir.dt.float32
F32R = mybir.dt.float32r
BF16 = mybir.dt.bfloat16

B, T, E = 4, 1024, 1024
H, D = 16, 64
TQ = T // 2          # query rows per core
N_CORES = 8
EC = E // 128        # e-chunks
SCALE = float(D) ** -0.5

_built = {}


def build_nc(dbg=False):
    nc = bacc.Bacc("TRN2", target_bir_lowering=False, debug=False,
                   num_devices=N_CORES)

    xT = nc.dram_tensor("xT", [E, T], F32, kind="ExternalInput")
    wqT = nc.dram_tensor("wqT", [E, E], F32, kind="ExternalInput")
    wkT = nc.dram_tensor("wkT", [E, E], F32, kind="ExternalInput")
    wvA = nc.dram_tensor("wvA", [E + 1, E], F32, kind="ExternalInput")
    woA = nc.dram_tensor("woA", [E + 1, E], F32, kind="ExternalInput")
    bqd = nc.dram_tensor("bq", [E], F32, kind="ExternalInput")
    bkd = nc.dram_tensor("bk", [E], F32, kind="ExternalInput")
    noiseT = nc.dram_tensor("noiseT", [H, T, TQ], F32, kind="ExternalInput")
    consts = nc.dram_tensor("consts", [18], F32, kind="ExternalInput")
    onesd = nc.dram_tensor("onesd", [128], F32, kind="ExternalInput")
    yD = nc.dram_tensor("y", [TQ, E], F32, kind="ExternalOutput")
    if dbg:
        qT_D = nc.dram_tensor("qT_dbg", [E, TQ], F32, kind="ExternalOutput")
        kT_D = nc.dram_tensor("kT_dbg", [E, T], F32, kind="ExternalOutput")
        v_D = nc.dram_tensor("v_dbg", [T, E], BF16, kind="ExternalOutput")
        p_D = nc.dram_tensor("p_dbg", [T, TQ], BF16, kind="ExternalOutput")
        m_D = nc.dram_tensor("m_dbg", [T, TQ], BF16, kind="ExternalOutput")
        den_D = nc.dram_tensor("den_dbg", [1, 2 * TQ], F32, kind="ExternalOutput")
        oT_D = nc.dram_tensor("oT_dbg", [E, TQ], F32, kind="ExternalOutput")
        rcb_D = nc.dram_tensor("rcb_dbg", [128, TQ], F32, kind="ExternalOutput")
        p5_D = nc.dram_tensor("p5_dbg", [T, TQ], BF16, kind="ExternalOutput")
        m5_D = nc.dram_tensor("m5_dbg", [T, TQ], BF16, kind="ExternalOutput")
        otr_D = nc.dram_tensor("otr_dbg", [64, TQ], F32, kind="ExternalOutput")
        den5_D = nc.dram_tensor("den5_dbg", [1, 2 * TQ], F32, kind="ExternalOutput")

    with tile.TileContext(nc) as tc:
        with tc.tile_pool(name="persist", bufs=1) as per, \
             tc.tile_pool(name="wst", bufs=2) as wst, \
             tc.tile_pool(name="wst1", bufs=2) as wst1:

            # ---- persistent tiles ----
            qT_sb = per.tile([128, EC, TQ], F32R)          # [d-tile, t] 2MB
            kT_sb = per.tile([128, EC, T], F32R)           # [d-tile, s] 4MB
            v_sb = per.tile([128, EC, E], BF16)           # [s-tile, d] 2MB
            oT_sb = per.tile([128, EC, TQ], F32R)          # [pair, t]   2MB
            ones_r = per.tile([1, 128], F32R)              # K=1 bias rows
            nc.sync.dma_start(ones_r[:], onesd.ap().bitcast(F32R)[None, :])
            ones_bf = per.tile([128, 1], BF16)            # rowsum lhsT
            nc.vector.memset(ones_bf[:], 1.0)
            cb = per.tile([128, 18], F32)                 # consts broadcast
            c_ap = consts.ap()
            nc.gpsimd.dma_start(
                out=cb[:],
                in_=bass.AP(tensor=c_ap.tensor, offset=c_ap.offset,
                            ap=[[0, 128]] + list(c_ap.ap)))
            bq_sb = per.tile([128, EC], F32)
            nc.sync.dma_start(bq_sb[:], bqd.ap().rearrange("(j p) -> p j", p=128))
            bk_sb = per.tile([128, EC], F32)
            nc.sync.dma_start(bk_sb[:], bkd.ap().rearrange("(j p) -> p j", p=128))

            # ================= Phase 1: projections =================
            # weights streamed as column-halves [e, 512] (16KB/partition);
            # xT scoped to this phase so its 32KB frees before attention.
            with tc.tile_pool(name="pps", bufs=4, space="PSUM") as pps, \
                 tc.tile_pool(name="xtp", bufs=1) as xtp:
                xT_sb = xtp.tile([128, EC, T], F32R)       # [e-chunk, t] 4MB
                nc.sync.dma_start(xT_sb[:], xT.ap().bitcast(F32R).rearrange("(c p) t -> p c t", p=128))

                # qT[d,t] : lhsT = wqT chunk [e,d-tile], rhs = xT[:, :TQ]
                for ch in range(2):
                    w_sb = wst.tile([128, EC, TQ], F32R, tag="w")
                    nc.sync.dma_start(
                        w_sb[:], wqT.ap().bitcast(F32R)[:, ch * TQ:(ch + 1) * TQ]
                        .rearrange("(c p) f -> p c f", p=128))
                    for jj in range(EC // 2):
                        j = ch * (EC // 2) + jj
                        ps = pps.tile([128, TQ], F32, tag="pp")
                        for ecs in range(EC):
                            nc.tensor.matmul(
                                ps[:], w_sb[:, ecs, jj * 128:(jj + 1) * 128],
                                xT_sb[:, ecs, 0:TQ],
                                start=(ecs == 0), stop=(ecs == EC - 1))
                        nc.scalar.activation(qT_sb[:, j, :], ps[:],
                                             mybir.ActivationFunctionType.Identity,
                                             bias=bq_sb[:, j:j + 1])

                # kT[d,s]
                for ch in range(2):
                    w_sb = wst.tile([128, EC, TQ], F32R, tag="w")
                    nc.sync.dma_start(
                        w_sb[:], wkT.ap().bitcast(F32R)[:, ch * TQ:(ch + 1) * TQ]
                        .rearrange("(c p) f -> p c f", p=128))
                    for jj in range(EC // 2):
                        j = ch * (EC // 2) + jj
                        for nh in range(2):
                            ps = pps.tile([128, TQ], F32, tag="pp")
                            for ecs in range(EC):
                                nc.tensor.matmul(
                                    ps[:], w_sb[:, ecs, jj * 128:(jj + 1) * 128],
                                    xT_sb[:, ecs, nh * TQ:(nh + 1) * TQ],
                                    start=(ecs == 0), stop=(ecs == EC - 1))
                            nc.scalar.activation(kT_sb[:, j, nh * TQ:(nh + 1) * TQ],
                                                 ps[:],
                                                 mybir.ActivationFunctionType.Identity,
                                                 bias=bk_sb[:, j:j + 1])

                # v[s,d] : lhsT = xT chunk [e, s-tile], rhs = wvT [e, d]
                wv_b = wst1.tile([1, E], F32R, tag="wb")
                nc.sync.dma_start(wv_b[:], wvA.ap().bitcast(F32R)[E:E + 1, :])
                for nh in range(2):
                    w_sb = wst.tile([128, EC, TQ], F32R, tag="w")
                    nc.sync.dma_start(
                        w_sb[:], wvA.ap().bitcast(F32R)[0:E, nh * TQ:(nh + 1) * TQ]
                        .rearrange("(c p) f -> p c f", p=128))
                    for i in range(EC):
                        ps = pps.tile([128, TQ], F32, tag="pp")
                        for ecs in range(EC):
                            nc.tensor.matmul(
                                ps[:], xT_sb[:, ecs, i * 128:(i + 1) * 128],
                                w_sb[:, ecs, :],
                                start=(ecs == 0), stop=False)
                        nc.tensor.matmul(
                            ps[:], ones_r[:],
                            wv_b[:, nh * TQ:(nh + 1) * TQ],
                            start=False, stop=True)
                        nc.scalar.activation(v_sb[:, i, nh * TQ:(nh + 1) * TQ], ps[:],
                                             mybir.ActivationFunctionType.Copy)

            if dbg:
                nc.sync.dma_start(qT_D.ap().bitcast(F32R).rearrange("(c p) t -> p c t", p=128), qT_sb[:])
                nc.sync.dma_start(kT_D.ap().bitcast(F32R).rearrange("(c p) t -> p c t", p=128), kT_sb[:])
                nc.sync.dma_start(v_D.ap().rearrange("(c p) t -> p c t", p=128), v_sb[:])

            # ================= Phase 2: attention =================
            with tc.tile_pool(name="noi", bufs=2) as noip, \
                 tc.tile_pool(name="pp_", bufs=3) as ppool, \
                 tc.tile_pool(name="mp_", bufs=3) as mpool, \
                 tc.tile_pool(name="pm_", bufs=3) as pmpool, \
                 tc.tile_pool(name="rcb", bufs=2) as rcbp, \
                 tc.tile_pool(name="denb", bufs=2) as denb, \
                 tc.tile_pool(name="sps", bufs=3, space="PSUM") as stps, \
                 tc.tile_pool(name="ops", bufs=2, space="PSUM") as otps, \
                 tc.tile_pool(name="ops2", bufs=2, space="PSUM") as otps2, \
                 tc.tile_pool(name="dps", bufs=1, space="PSUM") as denps:

                for pair in range(H // 2):
                    # separate PSUM banks per head: start=True clears a whole
                    # bank, so the two heads of a pair must not share one.
                    ot_e = otps.tile([64, TQ], F32, tag="ot")
                    ot_o = otps2.tile([128, TQ], F32, tag="ot2")
                    otp_h = (ot_e, ot_o)
                    den_pair = denb.tile([1, 2 * TQ], F32, tag="db")
                    rec_pair = denb.tile([1, 2 * TQ], F32, tag="rb")
                    for sub in range(2):
                        h = 2 * pair + sub
                        p0 = sub * 64
                        noi = noip.tile([128, EC, TQ], F32, tag="n")
                        nc.sync.dma_start(
                            noi[:], noiseT.ap()[h].rearrange("(c p) t -> p c t", p=128))
                        dnp = denps.tile([1, TQ], F32, tag="d")
                        for sc in range(EC):
                            st = stps.tile([128, TQ], F32, tag="s")
                            nc.tensor.matmul(
                                st[:],
                                kT_sb[p0:p0 + 64, pair, sc * 128:(sc + 1) * 128],
                                qT_sb[p0:p0 + 64, pair, :],
                                start=True, stop=True)
                            P = ppool.tile([128, TQ], BF16, tag="p")
                            nc.scalar.activation(P[:], st[:],
                                                 mybir.ActivationFunctionType.Exp,
                                                 bias=cb[:, h:h + 1], scale=SCALE)
                            M = mpool.tile([128, TQ], BF16, tag="m")
                            nc.vector.tensor_scalar(M[:], noi[:, sc, :],
                                                    cb[:, 16:17], None,
                                                    mybir.AluOpType.is_gt)
                            nc.tensor.matmul(dnp[:], ones_bf[:], P[:],
                                             start=(sc == 0), stop=(sc == EC - 1))
                            if dbg and h == 0:
                                nc.sync.dma_start(p_D.ap()[sc * 128:(sc + 1) * 128, :], P[:])
                                nc.sync.dma_start(m_D.ap()[sc * 128:(sc + 1) * 128, :], M[:])
                            if dbg and h == 5:
                                nc.sync.dma_start(p5_D.ap()[sc * 128:(sc + 1) * 128, :], P[:])
                                nc.sync.dma_start(m5_D.ap()[sc * 128:(sc + 1) * 128, :], M[:])
                            Pm = pmpool.tile([128, TQ], BF16, tag="q")
                            nc.vector.tensor_tensor(Pm[:], P[:], M[:],
                                                    mybir.AluOpType.mult)
                            ot_dst = otp_h[sub][p0:p0 + 64, :] if sub else otp_h[0][:]
                            nc.tensor.matmul(
                                ot_dst,
                                v_sb[:, sc, h * 64:(h + 1) * 64],
                                Pm[:],
                                start=(sc == 0), stop=(sc == EC - 1))
                        # den*(1-Tm+eps), fused with the PSUM->SBUF copy
                        nc.vector.tensor_scalar(
                            den_pair[:, sub * TQ:(sub + 1) * TQ], dnp[:],
                            cb[0:1, 17:18], None, mybir.AluOpType.mult)

                    # finalize pair: oT = otp * 1/(den*(1-Tm+eps))
                    nc.vector.reciprocal(rec_pair[:], den_pair[:])
                    # partition_broadcast only writes base-0 output; broadcast
                    # each head to a full-128 tile, use the matching half.
                    rcb_e = rcbp.tile([128, TQ], F32, tag="r")
                    nc.gpsimd.partition_broadcast(rcb_e[:], rec_pair[:, 0:TQ])
                    rcb_o = rcbp.tile([128, TQ], F32, tag="r2")
                    nc.gpsimd.partition_broadcast(rcb_o[:], rec_pair[:, TQ:2 * TQ])
                    nc.vector.tensor_tensor(oT_sb[0:64, pair, :], ot_e[:],
                                            rcb_e[0:64, :], mybir.AluOpType.mult)
                    nc.vector.tensor_tensor(oT_sb[64:128, pair, :], ot_o[64:128, :],
                                            rcb_o[64:128, :], mybir.AluOpType.mult)
                    if dbg and pair == 0:
                        nc.sync.dma_start(den_D.ap(), den_pair[:])
                    if dbg and pair == 2:
                        otr_sb = rcbp.tile([128, TQ], F32, tag="dbgo")
                        nc.vector.tensor_copy(otr_sb[64:128, :], ot_o[64:128, :])
                        nc.sync.dma_start(otr_D.ap(), otr_sb[64:128, :])
                        nc.sync.dma_start(den5_D.ap(), den_pair[:])
                        nc.sync.dma_start(rcb_D.ap()[0:64], rcb_e[0:64, :])
                        nc.sync.dma_start(rcb_D.ap()[64:128], rcb_o[64:128, :])

            if dbg:
                nc.sync.dma_start(oT_D.ap().bitcast(F32R).rearrange("(c p) t -> p c t", p=128), oT_sb[:])

            # ================= Phase 3: out projection =================
            with tc.tile_pool(name="yps", bufs=3, space="PSUM") as yps, \
                 tc.tile_pool(name="ysb", bufs=3) as ysbp:
                wo_b = wst1.tile([1, E], F32R, tag="wb")
                nc.sync.dma_start(wo_b[:], woA.ap().bitcast(F32R)[E:E + 1, :])
                for nh in range(2):
                    w_sb = wst.tile([128, EC, TQ], F32R, tag="w")
                    nc.sync.dma_start(
                        w_sb[:], woA.ap().bitcast(F32R)[0:E, nh * TQ:(nh + 1) * TQ]
                        .rearrange("(c p) f -> p c f", p=128))
                    for tt in range(TQ // 128):
                        ps = yps.tile([128, TQ], F32, tag="y")
                        for j in range(EC):
                            nc.tensor.matmul(
                                ps[:], oT_sb[:, j, tt * 128:(tt + 1) * 128],
                                w_sb[:, j, :],
                                start=(j == 0), stop=False)
                        nc.tensor.matmul(ps[:], ones_r[:],
                                         wo_b[:, nh * TQ:(nh + 1) * TQ],
                                         start=False, stop=True)
                        ysb = ysbp.tile([128, TQ], F32, tag="ys")
                        nc.scalar.activation(ysb[:], ps[:],
                                             mybir.ActivationFunctionType.Copy)
                        nc.sync.dma_start(
                            yD.ap()[tt * 128:(tt + 1) * 128, nh * TQ:(nh + 1) * TQ],
                            ysb[:])

    nc.compile()
    return nc


def get_nc():
    if "nc" not in _built:
        _built["nc"] = build_nc()
    return _built["nc"]


def _host_consts(theta, corr_w):
    """theta-derived scalars, replicating the reference's fp32 math."""
    try:
        import jax
        import jax.numpy as jnp
        with jax.default_device(jax.devices("cpu")[0]):
            th = jax.nn.sigmoid(jnp.asarray(theta)) * (jnp.pi / 2)
            orders = jnp.arange(1, 5)
            ang = 2.0 * orders[:, None].astype(th.dtype) * th[None, :]
            Qk = jnp.where((orders % 2 == 1)[:, None], jnp.sin(ang), jnp.cos(ang))
            bias = 0.1 * jnp.einsum("k,kh->h", jnp.asarray(corr_w)[1:], Qk)
            t_mean = jnp.mean(jnp.abs(jnp.sin(2.0 * th)))
            bias = np.asarray(bias, np.float32)
            t_mean = np.float32(t_mean)
    except Exception:
        th = (1.0 / (1.0 + np.exp(-np.asarray(theta, np.float32)))) * np.float32(np.pi / 2)
        orders = np.arange(1, 5, dtype=np.float32)
        ang = np.float32(2.0) * orders[:, None] * th[None, :]
        Qk = np.where((orders.astype(np.int32) % 2 == 1)[:, None],
                      np.sin(ang, dtype=np.float32), np.cos(ang, dtype=np.float32))
        bias = np.float32(0.1) * (np.asarray(corr_w, np.float32)[1:] @ Qk)
        t_mean = np.mean(np.abs(np.sin(np.float32(2.0) * th, dtype=np.float32)),
                         dtype=np.float32)
    c = np.float32(1.0) - t_mean + np.float32(1e-8)
    return np.concatenate([bias.astype(np.float32),
                           np.array([t_mean, c], np.float32)])


def build_in_maps(inputs):
    return _build_in_maps(**inputs)


def _build_in_maps(x, noise, Wq, bq, Wk, bk, Wv, bv, Wo, bo, theta, corr_w):
    x = np.asarray(x, np.float32)
    noise = np.asarray(noise, np.float32)

    wqT = np.ascontiguousarray(np.asarray(Wq, np.float32).T)
    wkT = np.ascontiguousarray(np.asarray(Wk, np.float32).T)
    wvA = np.ascontiguousarray(
        np.vstack([np.asarray(Wv, np.float32).T, np.asarray(bv, np.float32)[None]]))
    woA = np.ascontiguousarray(
        np.vstack([np.asarray(Wo, np.float32).T, np.asarray(bo, np.float32)[None]]))
    consts = _host_consts(theta, corr_w)

    in_maps = []
    for core in range(N_CORES):
        b, th_ = core // 2, core % 2
        t0 = th_ * TQ
        xT = np.ascontiguousarray(x[b].T)  # [E, T]
        if th_ == 1:  # core's own query columns first (s-permutation)
            xT = np.ascontiguousarray(np.concatenate([xT[:, TQ:], xT[:, :TQ]], axis=1))
        nT = noise[b].transpose(0, 2, 1)[:, :, t0:t0 + TQ]  # [H, s, t-slice]
        if th_ == 1:
            nT = np.concatenate([nT[:, TQ:, :], nT[:, :TQ, :]], axis=1)
        nT = np.ascontiguousarray(nT)
        in_maps.append({
            "xT": xT, "wqT": wqT, "wkT": wkT, "wvA": wvA, "woA": woA,
            "bq": np.asarray(bq, np.float32), "bk": np.asarray(bk, np.float32),
            "noiseT": nT, "consts": consts,
            "onesd": np.ones(128, np.float32),
        })
    return in_maps


def kernel(x, noise, Wq, bq, Wk, bk, Wv, bv, Wo, bo, theta, corr_w):
    nc = get_nc()
    in_maps = _build_in_maps(x, noise, Wq, bq, Wk, bk, Wv, bv, Wo, bo,
                             theta, corr_w)
    res = run_bass_kernel_spmd(nc, in_maps, core_ids=list(range(N_CORES)))

    out = np.empty((B, T, E), np.float32)
    for core in range(N_CORES):
        b, th_ = core // 2, core % 2
        out[b, th_ * TQ:(th_ + 1) * TQ, :] = res.results[core]["y"]
    return out
